# Initial kernel scaffold
#
"""Trainium2 Bass kernel for causal multi-head attention (16 heads, D=1024).

Sharding: tensor-parallel over heads. Each of the 8 cores owns 2 heads:
Wq/Wk/Wv split column-wise (128 cols per core), Wo split row-wise
(128 rows per core). Each core computes a full-shape partial of the
output projection; the all-reduce over partials (+ bias) happens on the
host during unsharding.

Device-side layout trick: everything is computed transposed.
  qT, kT = (x @ Wq_c)^T, (x @ Wk_c)^T          -> [128=2*Dh, B*S]
  scoresT[k, q] = kT.T-tile @ qT               -> k on partitions
  exp on ScalarE (no max subtraction needed: |scores/8| <~ 4)
  ctxT[dh, q] accumulated via lhsT=v_natural (v obtained by PE transpose)
  denominators via ones-column matmuls; normalize via PE broadcast
  outT_partial = Wo_c^T-chunk @ ctxT           -> [1024, B*S]
"""

import sys

import numpy as np

sys.path.insert(0, "/opt/trn_rl_repo")

B, S, D = 4, 2048, 1024
H, DH = 16, 64
NCORES = 8
HPC = H // NCORES            # heads per core = 2
BS = B * S                   # 8192 flattened tokens
QB = 512                     # query block (free dim of scores matmuls)
NBLK = BS // QB              # 16 projection blocks
KT = D // 128                # 8 contraction tiles for projections
NQB = S // QB                # 4 query blocks per batch
NKT = S // 128               # 16 key tiles per batch

_BUILT = None                # cached compiled Bass module
LAST_RESULTS = None          # BassKernelResults from the last run


def _emit(tc, outT, xT, wq, wk, wv, wo, dmask, ident, ones):
    from contextlib import ExitStack

    import concourse.tile as tile  # noqa: F401
    from concourse import mybir

    F32R = mybir.dt.float32r
    F32 = mybir.dt.float32
    Exp = mybir.ActivationFunctionType.Exp

    nc = tc.nc
    with ExitStack() as ctx:
        consts = ctx.enter_context(tc.tile_pool(name="consts", bufs=1))
        persist = ctx.enter_context(tc.tile_pool(name="persist", bufs=1))
        xpool = ctx.enter_context(tc.tile_pool(name="xpool", bufs=2))
        expp = ctx.enter_context(tc.tile_pool(name="expp", bufs=3))
        ctxp = ctx.enter_context(tc.tile_pool(name="ctxp", bufs=2))
        misc = ctx.enter_context(tc.tile_pool(name="misc", bufs=2))
        psA = ctx.enter_context(tc.tile_pool(name="psA", bufs=2, space="PSUM"))
        psB = ctx.enter_context(tc.tile_pool(name="psB", bufs=2, space="PSUM"))
        psC = ctx.enter_context(tc.tile_pool(name="psC", bufs=1, space="PSUM"))

        # ---- constants into SBUF ----
        wq_sb = consts.tile([128, KT * 128], F32R)
        wk_sb = consts.tile([128, KT * 128], F32R)
        wv_sb = consts.tile([128, KT * 128], F32R)
        wo_sb = consts.tile([128, D], F32R)
        dmask_sb = consts.tile([128, 128], F32R)
        ident_sb = consts.tile([128, DH], F32R)
        ones_sb = consts.tile([128, DH], F32R)
        wq_v = wq.rearrange("(k p) c -> k p c", p=128)
        wk_v = wk.rearrange("(k p) c -> k p c", p=128)
        wv_v = wv.rearrange("(k p) c -> k p c", p=128)
        for kt in range(KT):
            nc.sync.dma_start(wq_sb[:, kt * 128:(kt + 1) * 128], wq_v[kt])
            nc.sync.dma_start(wk_sb[:, kt * 128:(kt + 1) * 128], wk_v[kt])
            nc.sync.dma_start(wv_sb[:, kt * 128:(kt + 1) * 128], wv_v[kt])
        wo_v = wo.rearrange("p (c n) -> p c n", n=128)  # wo is [128, 1024]
        nc.sync.dma_start(wo_sb[:], wo)
        nc.sync.dma_start(dmask_sb[:], dmask)
        nc.sync.dma_start(ident_sb[:], ident)
        nc.sync.dma_start(ones_sb[:], ones)

        # ---- persistent activations ----
        qT_sb = persist.tile([128, BS], F32R)   # [2*DH, B*S]
        kT_sb = persist.tile([128, BS], F32R)
        # v natural: slot (h*64+g) holds rows g*128..g*128+128 of head h's V
        v_sb = persist.tile([128, HPC * (BS // 128) * DH], F32R)

        xT_v = xT.rearrange("(k p) n -> k p n", p=128)

        # ================= phase 1: projections =================
        for blk in range(NBLK):
            cols = slice(blk * QB, (blk + 1) * QB)
            xblk = xpool.tile([128, KT * QB], F32R)
            for kt in range(KT):
                nc.sync.dma_start(xblk[:, kt * QB:(kt + 1) * QB], xT_v[kt, :, cols])

            for w_sb, dst in ((wq_sb, qT_sb), (wk_sb, kT_sb)):
                ps = psB.tile([128, QB], F32, tag="mm")
                for kt in range(KT):
                    nc.tensor.matmul(
                        ps[:],
                        w_sb[:, kt * 128:(kt + 1) * 128],
                        xblk[:, kt * QB:(kt + 1) * QB],
                        start=(kt == 0),
                        stop=(kt == KT - 1),
                    )
                nc.vector.tensor_copy(dst[:, cols], ps[:])

            # V: project then transpose to natural [seq, dh] tiles
            ps = psB.tile([128, QB], F32, tag="mm")
            for kt in range(KT):
                nc.tensor.matmul(
                    ps[:],
                    wv_sb[:, kt * 128:(kt + 1) * 128],
                    xblk[:, kt * QB:(kt + 1) * QB],
                    start=(kt == 0),
                    stop=(kt == KT - 1),
                )
            vtmp = misc.tile([128, QB], F32R, tag="vtmp")
            nc.vector.tensor_copy(vtmp[:], ps[:])
            for h in range(HPC):
                hp = slice(h * DH, (h + 1) * DH)
                for cc in range(QB // 128):
                    pst = psB.tile([128, DH], F32R, tag="tr")
                    nc.tensor.transpose(
                        pst[:], vtmp[hp, cc * 128:(cc + 1) * 128], ident_sb[hp, :]
                    )
                    g = blk * (QB // 128) + cc  # global 128-row tile index
                    s0 = (h * (BS // 128) + g) * DH
                    nc.vector.tensor_copy(v_sb[:, s0:s0 + DH], pst[:])

        # ================= phase 2: attention + out-proj =================
        for b in range(B):
            for jq in range(NQB):
                nkt = 4 * (jq + 1)          # causal: valid key tiles
                qc = b * S + jq * QB        # query col offset in qT_sb
                ps_ctx = psC.tile([128, QB], F32, tag="ctx")
                ps_den = psC.tile([128, QB], F32, tag="den")
                for t in range(nkt):
                    kc = (b * NKT + t) * 128  # key col offset in kT_sb
                    ps_s = psA.tile([128, 2 * QB], F32)
                    for h in range(HPC):
                        hp = slice(h * DH, (h + 1) * DH)
                        nc.tensor.matmul(
                            ps_s[:, h * QB:(h + 1) * QB],
                            kT_sb[hp, kc:kc + 128],
                            qT_sb[hp, qc:qc + QB],
                            start=True,
                            stop=True,
                        )
                    ex = expp.tile([128, 2 * QB], F32R)
                    tt = t - 4 * jq
                    if tt < 0:
                        nc.scalar.activation(ex[:], ps_s[:], Exp)
                        c0 = 0
                    else:
                        c0 = 128 * tt
                        for h in range(HPC):
                            o = h * QB
                            nc.scalar.activation(
                                ex[:, o + c0:o + QB], ps_s[:, o + c0:o + QB], Exp
                            )
                            nc.vector.tensor_mul(
                                ex[:, o + c0:o + c0 + 128],
                                ex[:, o + c0:o + c0 + 128],
                                dmask_sb[:],
                            )
                    st, sp = (t == 0), (t == nkt - 1)
                    for h in range(HPC):
                        o = h * QB
                        s0 = (h * (BS // 128) + b * NKT + t) * DH
                        nc.tensor.matmul(
                            ps_ctx[h * DH:(h + 1) * DH, c0:QB],
                            v_sb[:, s0:s0 + DH],
                            ex[:, o + c0:o + QB],
                            start=st,
                            stop=sp,
                            skip_group_check=True,
                        )
                        nc.tensor.matmul(
                            ps_den[h * DH:h * DH + 1, c0:QB],
                            ones_sb[:, 0:1],
                            ex[:, o + c0:o + QB],
                            start=st,
                            stop=sp,
                            skip_group_check=True,
                        )

                # normalize: ctx * (1/den) with PE partition-broadcast
                den_sb = misc.tile([128, QB], F32R, tag="den")
                ps_sc = psB.tile([128, QB], F32, tag="mm")
                for h in range(HPC):
                    r = h * DH
                    nc.vector.reciprocal(den_sb[r:r + 1, :], ps_den[r:r + 1, :])
                    nc.tensor.matmul(
                        ps_sc[r:r + DH, :],
                        ones_sb[r:r + 1, 0:DH],
                        den_sb[r:r + 1, :],
                        start=True,
                        stop=True,
                        skip_group_check=True,
                    )
                sc_sb = misc.tile([128, QB], F32, tag="sc")
                nc.scalar.copy(sc_sb[:], ps_sc[:])
                cx = ctxp.tile([128, QB], F32R)
                nc.vector.tensor_mul(cx[:], ps_ctx[:], sc_sb[:])

                # out projection: outT[ch*128:+128, qc:qc+QB] = Wo_c^T @ ctx
                for ch in range(D // 128):
                    ps_o = psB.tile([128, QB], F32, tag="mm")
                    nc.tensor.matmul(
                        ps_o[:], wo_sb[:, ch * 128:(ch + 1) * 128], cx[:],
                        start=True, stop=True,
                    )
                    ob = misc.tile([128, QB], F32R, tag="out")
                    nc.vector.tensor_copy(ob[:], ps_o[:])
                    nc.sync.dma_start(outT[ch * 128:(ch + 1) * 128, qc:qc + QB], ob[:])
    _ = wo_v


def _build():
    global _BUILT
    if _BUILT is not None:
        return _BUILT
    import concourse.tile as tile
    from concourse import bacc, mybir

    F32R = mybir.dt.float32r

    nc = bacc.Bacc(
        "TRN2",
        target_bir_lowering=False,
        debug=False,
        enable_asserts=False,
        num_devices=NCORES,
    )
    xT = nc.dram_tensor("xT", [D, BS], F32R, kind="ExternalInput").ap()
    wq = nc.dram_tensor("wq", [D, 128], F32R, kind="ExternalInput").ap()
    wk = nc.dram_tensor("wk", [D, 128], F32R, kind="ExternalInput").ap()
    wv = nc.dram_tensor("wv", [D, 128], F32R, kind="ExternalInput").ap()
    wo = nc.dram_tensor("wo", [128, D], F32R, kind="ExternalInput").ap()
    dmask = nc.dram_tensor("dmask", [128, 128], F32R, kind="ExternalInput").ap()
    ident = nc.dram_tensor("ident", [128, DH], F32R, kind="ExternalInput").ap()
    ones = nc.dram_tensor("ones", [128, DH], F32R, kind="ExternalInput").ap()
    outT = nc.dram_tensor("outT", [D, BS], F32R, kind="ExternalOutput").ap()

    with tile.TileContext(nc) as tc:
        with nc.allow_low_precision(reason="float32r carries fp32 bits"):
            _emit(tc, outT, xT, wq, wk, wv, wo, dmask, ident, ones)
    nc.compile()
    _BUILT = nc
    return nc


def _host_inputs(x, Wq, Wk, Wv, Wo):
    """Shard + lay out the full inputs for the 8 cores."""
    x2 = np.ascontiguousarray(x.reshape(BS, D).T, dtype=np.float32)
    dmask = (np.arange(128)[None, :] >= np.arange(128)[:, None]).astype(np.float32)
    ident = np.tile(np.eye(DH, dtype=np.float32), (2, 1))
    ones = np.ones((128, DH), dtype=np.float32)
    in_maps = []
    for c in range(NCORES):
        cs = slice(c * HPC * DH, (c + 1) * HPC * DH)
        in_maps.append({
            "xT": x2,
            # fold the 1/sqrt(DH) score scale into Wq
            "wq": np.ascontiguousarray(Wq[:, cs], dtype=np.float32) / np.sqrt(DH),
            "wk": np.ascontiguousarray(Wk[:, cs], dtype=np.float32),
            "wv": np.ascontiguousarray(Wv[:, cs], dtype=np.float32),
            "wo": np.ascontiguousarray(Wo[cs, :], dtype=np.float32),
            "dmask": dmask,
            "ident": ident,
            "ones": ones,
        })
    return in_maps


def kernel(x, Wq, Wk, Wv, Wo, bo):
    global LAST_RESULTS
    from concourse.bass_utils import run_bass_kernel_spmd

    nc = _build()
    in_maps = _host_inputs(
        np.asarray(x), np.asarray(Wq), np.asarray(Wk), np.asarray(Wv), np.asarray(Wo)
    )
    res = run_bass_kernel_spmd(nc, in_maps, core_ids=list(range(NCORES)))
    LAST_RESULTS = res
    acc = np.zeros((D, BS), dtype=np.float32)
    for r in res.results:
        acc += r["outT"]
    out = acc.T + np.asarray(bo, dtype=np.float32)[None, :]
    return out.reshape(B, S, D).astype(np.float32)


# revision 10
# speedup vs baseline: 1.0234x; 1.0234x over previous
"""Trainium2 Bass kernel for causal multi-head attention (16 heads, D=1024).

Sharding: tensor-parallel over heads. Each of the 8 cores owns 2 heads:
Wq/Wk/Wv split column-wise (128 cols per core), Wo split row-wise
(128 rows per core). Each core computes a full-shape partial of the
output projection; the all-reduce over partials (+ bias) happens on the
host during unsharding.

Device-side layout trick: everything is computed transposed.
  qT, kT = (x @ Wq_c)^T, (x @ Wk_c)^T          -> [128=2*Dh, B*S]
  scoresT[k, q] = kT.T-tile @ qT               -> k on partitions
  exp on ScalarE (no max subtraction needed: |scores/8| <~ 4)
  ctxT[dh, q] accumulated via lhsT=v_natural (v obtained by PE transpose)
  denominators via ones-column matmuls; normalize via PE broadcast
  outT_partial = Wo_c^T-chunk @ ctxT           -> [1024, B*S]
"""

import sys

import numpy as np

sys.path.insert(0, "/opt/trn_rl_repo")

B, S, D = 4, 2048, 1024
H, DH = 16, 64
NCORES = 8
HPC = H // NCORES            # heads per core = 2
BS = B * S                   # 8192 flattened tokens
QB = 512                     # query block (free dim of scores matmuls)
NBLK = BS // QB              # 16 projection blocks
KT = D // 128                # 8 contraction tiles for projections
NQB = S // QB                # 4 query blocks per batch
NKT = S // 128               # 16 key tiles per batch

_BUILT = None                # cached compiled Bass module
LAST_RESULTS = None          # BassKernelResults from the last run


def _emit(tc, outT, xT, wq, wk, wv, wo, dmask, ident, ones):
    from contextlib import ExitStack

    import concourse.tile as tile  # noqa: F401
    from concourse import mybir

    F32R = mybir.dt.float32r
    F32 = mybir.dt.float32
    Exp = mybir.ActivationFunctionType.Exp

    nc = tc.nc
    with ExitStack() as ctx:
        consts = ctx.enter_context(tc.tile_pool(name="consts", bufs=1))
        persist = ctx.enter_context(tc.tile_pool(name="persist", bufs=1))
        xpool = ctx.enter_context(tc.tile_pool(name="xpool", bufs=2))
        expp = ctx.enter_context(tc.tile_pool(name="expp", bufs=3))
        ctxp = ctx.enter_context(tc.tile_pool(name="ctxp", bufs=2))
        misc = ctx.enter_context(tc.tile_pool(name="misc", bufs=2))
        psA = ctx.enter_context(tc.tile_pool(name="psA", bufs=2, space="PSUM"))
        psB = ctx.enter_context(tc.tile_pool(name="psB", bufs=2, space="PSUM"))
        psC = ctx.enter_context(tc.tile_pool(name="psC", bufs=1, space="PSUM"))

        # ---- constants into SBUF ----
        wq_sb = consts.tile([128, KT * 128], F32R)
        wk_sb = consts.tile([128, KT * 128], F32R)
        wv_sb = consts.tile([128, KT * 128], F32R)
        wo_sb = consts.tile([128, D], F32R)
        dmask_sb = consts.tile([128, 128], F32R)
        ident_sb = consts.tile([128, DH], F32R)
        ones_sb = consts.tile([128, DH], F32R)
        wq_v = wq.rearrange("(k p) c -> k p c", p=128)
        wk_v = wk.rearrange("(k p) c -> k p c", p=128)
        wv_v = wv.rearrange("(k p) c -> k p c", p=128)
        for kt in range(KT):
            nc.sync.dma_start(wq_sb[:, kt * 128:(kt + 1) * 128], wq_v[kt])
            nc.sync.dma_start(wk_sb[:, kt * 128:(kt + 1) * 128], wk_v[kt])
            nc.sync.dma_start(wv_sb[:, kt * 128:(kt + 1) * 128], wv_v[kt])
        wo_v = wo.rearrange("p (c n) -> p c n", n=128)  # wo is [128, 1024]
        nc.sync.dma_start(wo_sb[:], wo)
        nc.sync.dma_start(dmask_sb[:], dmask)
        nc.sync.dma_start(ident_sb[:], ident)
        nc.sync.dma_start(ones_sb[:], ones)

        # ---- persistent activations ----
        qT_sb = persist.tile([128, BS], F32R)   # [2*DH, B*S]
        kT_sb = persist.tile([128, BS], F32R)
        # v natural + ones column: slot (h*64+g) holds [v_tile | 1] for rows
        # g*128..g*128+128 of head h's V -> 65 cols per slot
        VW = DH + 1
        v_sb = persist.tile([128, HPC * (BS // 128) * VW], F32R)
        ones_cols = v_sb.rearrange("p (s c) -> p s c", c=VW)[:, :, DH:DH + 1]
        nc.vector.memset(ones_cols.bitcast(F32), 1.0)

        xT_v = xT.rearrange("(k p) n -> k p n", p=128)

        # ================= phase 1: projections =================
        for blk in range(NBLK):
            cols = slice(blk * QB, (blk + 1) * QB)
            xblk = xpool.tile([128, KT * QB], F32R)
            for kt in range(KT):
                nc.sync.dma_start(xblk[:, kt * QB:(kt + 1) * QB], xT_v[kt, :, cols])

            for w_sb, dst in ((wq_sb, qT_sb), (wk_sb, kT_sb)):
                ps = psB.tile([128, QB], F32, tag="b")
                for kt in range(KT):
                    nc.tensor.matmul(
                        ps[:],
                        w_sb[:, kt * 128:(kt + 1) * 128],
                        xblk[:, kt * QB:(kt + 1) * QB],
                        start=(kt == 0),
                        stop=(kt == KT - 1),
                    )
                nc.vector.tensor_copy(dst[:, cols], ps[:])

            # V: project then transpose to natural [seq, dh] tiles
            ps = psB.tile([128, QB], F32, tag="b")
            for kt in range(KT):
                nc.tensor.matmul(
                    ps[:],
                    wv_sb[:, kt * 128:(kt + 1) * 128],
                    xblk[:, kt * QB:(kt + 1) * QB],
                    start=(kt == 0),
                    stop=(kt == KT - 1),
                )
            vtmp = misc.tile([128, QB], F32R, tag="vtmp")
            nc.vector.tensor_copy(vtmp[:], ps[:])
            for h in range(HPC):
                hp = slice(h * DH, (h + 1) * DH)
                for cc in range(QB // 128):
                    pst = psB.tile([128, DH], F32R, tag="b")
                    nc.tensor.transpose(
                        pst[:], vtmp[hp, cc * 128:(cc + 1) * 128], ident_sb[hp, :]
                    )
                    g = blk * (QB // 128) + cc  # global 128-row tile index
                    s0 = (h * (BS // 128) + g) * VW
                    nc.vector.tensor_copy(v_sb[:, s0:s0 + DH], pst[:])

        # ================= phase 2: attention + out-proj =================
        for b in range(B):
            for jq in range(NQB):
                nkt = 4 * (jq + 1)          # causal: valid key tiles
                qc = b * S + jq * QB        # query col offset in qT_sb
                ps_ctx = [
                    psC.tile([DH + 1, QB], F32, tag=f"ctx{h}", name=f"ps_ctx{h}")
                    for h in range(HPC)
                ]
                for t in range(nkt):
                    kc = (b * NKT + t) * 128  # key col offset in kT_sb
                    ps_s = psA.tile([128, 2 * QB], F32)
                    for h in range(HPC):
                        hp = slice(h * DH, (h + 1) * DH)
                        nc.tensor.matmul(
                            ps_s[:, h * QB:(h + 1) * QB],
                            kT_sb[hp, kc:kc + 128],
                            qT_sb[hp, qc:qc + QB],
                            start=True,
                            stop=True,
                        )
                    ex = expp.tile([128, 2 * QB], F32R)
                    tt = t - 4 * jq
                    if tt < 0:
                        nc.scalar.activation(ex[:], ps_s[:], Exp)
                        c0 = 0
                    else:
                        c0 = 128 * tt
                        for h in range(HPC):
                            o = h * QB
                            nc.scalar.activation(
                                ex[:, o + c0:o + QB], ps_s[:, o + c0:o + QB], Exp
                            )
                            nc.vector.tensor_mul(
                                ex[:, o + c0:o + c0 + 128],
                                ex[:, o + c0:o + c0 + 128],
                                dmask_sb[:],
                            )
                    st, sp = (t == 0), (t == nkt - 1)
                    for h in range(HPC):
                        o = h * QB
                        s0 = (h * (BS // 128) + b * NKT + t) * VW
                        nc.tensor.matmul(
                            ps_ctx[h][:, c0:QB],
                            v_sb[:, s0:s0 + VW],
                            ex[:, o + c0:o + QB],
                            start=st,
                            stop=sp,
                            skip_group_check=True,
                        )

                # normalize: ctx * (1/den) with PE partition-broadcast.
                # Head 0 lands directly in cx2h rows 0:64; head 1 is staged at
                # partitions 0:64 and DMA'd into rows 64:128.
                cx2h = ctxp.tile([128, QB], F32R)
                for h in range(HPC):
                    den_sb = misc.tile([128, QB], F32R, tag="den")
                    nc.vector.reciprocal(den_sb[DH:DH + 1, :], ps_ctx[h][DH:DH + 1, :])
                    ps_sc = psB.tile([64, QB], F32, tag="b")
                    nc.tensor.matmul(
                        ps_sc[:],
                        ones_sb[DH:DH + 1, 0:DH],
                        den_sb[DH:DH + 1, :],
                        start=True,
                        stop=True,
                        skip_group_check=True,
                    )
                    sc_sb = misc.tile([64, QB], F32, tag="sc")
                    nc.scalar.copy(sc_sb[:], ps_sc[:])
                    if h == 0:
                        nc.vector.tensor_mul(
                            cx2h[0:DH, :], ps_ctx[h][0:DH, :], sc_sb[:]
                        )
                    else:
                        cxs = misc.tile([64, QB], F32R, tag="cxs")
                        nc.vector.tensor_mul(cxs[:], ps_ctx[h][0:DH, :], sc_sb[:])
                        nc.sync.dma_start(cx2h[DH:2 * DH, :], cxs[:])

                # out projection: outT[ch*128:+128, qc:qc+QB] = Wo_c^T @ ctx
                for ch in range(D // 128):
                    ps_o = psB.tile([128, QB], F32, tag="b")
                    nc.tensor.matmul(
                        ps_o[:], wo_sb[:, ch * 128:(ch + 1) * 128], cx2h[:],
                        start=True, stop=True,
                    )
                    ob = misc.tile([128, QB], F32R, tag="out")
                    nc.vector.tensor_copy(ob[:], ps_o[:])
                    nc.sync.dma_start(outT[ch * 128:(ch + 1) * 128, qc:qc + QB], ob[:])
    _ = wo_v


def _build(loop_n=None):
    global _BUILT
    if loop_n is None and _BUILT is not None:
        return _BUILT
    import concourse.tile as tile
    from concourse import bacc, mybir

    F32R = mybir.dt.float32r

    nc = bacc.Bacc(
        "TRN2",
        target_bir_lowering=False,
        debug=False,
        enable_asserts=False,
        num_devices=NCORES,
    )
    xT = nc.dram_tensor("xT", [D, BS], F32R, kind="ExternalInput").ap()
    wq = nc.dram_tensor("wq", [D, 128], F32R, kind="ExternalInput").ap()
    wk = nc.dram_tensor("wk", [D, 128], F32R, kind="ExternalInput").ap()
    wv = nc.dram_tensor("wv", [D, 128], F32R, kind="ExternalInput").ap()
    wo = nc.dram_tensor("wo", [128, D], F32R, kind="ExternalInput").ap()
    dmask = nc.dram_tensor("dmask", [128, 128], F32R, kind="ExternalInput").ap()
    ident = nc.dram_tensor("ident", [128, DH], F32R, kind="ExternalInput").ap()
    ones = nc.dram_tensor("ones", [128, DH], F32R, kind="ExternalInput").ap()
    outT = nc.dram_tensor("outT", [D, BS], F32R, kind="ExternalOutput").ap()

    with tile.TileContext(nc) as tc:
        with nc.allow_low_precision(reason="float32r carries fp32 bits"):
            if loop_n is None:
                _emit(tc, outT, xT, wq, wk, wv, wo, dmask, ident, ones)
            else:
                with tc.For_i(0, loop_n, 1):
                    _emit(tc, outT, xT, wq, wk, wv, wo, dmask, ident, ones)
    nc.compile()
    if loop_n is None:
        _BUILT = nc
    return nc


def _host_inputs(x, Wq, Wk, Wv, Wo):
    """Shard + lay out the full inputs for the 8 cores."""
    x2 = np.ascontiguousarray(x.reshape(BS, D).T, dtype=np.float32)
    dmask = (np.arange(128)[None, :] >= np.arange(128)[:, None]).astype(np.float32)
    ident = np.tile(np.eye(DH, dtype=np.float32), (2, 1))
    ones = np.ones((128, DH), dtype=np.float32)
    in_maps = []
    for c in range(NCORES):
        cs = slice(c * HPC * DH, (c + 1) * HPC * DH)
        in_maps.append({
            "xT": x2,
            # fold the 1/sqrt(DH) score scale into Wq
            "wq": np.ascontiguousarray(Wq[:, cs], dtype=np.float32) / np.sqrt(DH),
            "wk": np.ascontiguousarray(Wk[:, cs], dtype=np.float32),
            "wv": np.ascontiguousarray(Wv[:, cs], dtype=np.float32),
            "wo": np.ascontiguousarray(Wo[cs, :], dtype=np.float32),
            "dmask": dmask,
            "ident": ident,
            "ones": ones,
        })
    return in_maps


def kernel(x, Wq, Wk, Wv, Wo, bo):
    global LAST_RESULTS
    from concourse.bass_utils import run_bass_kernel_spmd

    nc = _build()
    in_maps = _host_inputs(
        np.asarray(x), np.asarray(Wq), np.asarray(Wk), np.asarray(Wv), np.asarray(Wo)
    )
    res = run_bass_kernel_spmd(nc, in_maps, core_ids=list(range(NCORES)))
    LAST_RESULTS = res
    acc = np.zeros((D, BS), dtype=np.float32)
    for r in res.results:
        acc += r["outT"]
    out = acc.T + np.asarray(bo, dtype=np.float32)[None, :]
    return out.reshape(B, S, D).astype(np.float32)
